# revision 1
# baseline (speedup 1.0000x reference)
"""DenseGATv2 layer on 8 Trainium2 NeuronCores (Bass/Tile).

Math: the reference computes, per head,
    e[i,j]  = leaky_relu(s_i[i] + s_j[j], 0.2)   (s_i = h@a_src, s_j = h@a_dst)
    attn    = softmax_j(where(adj[i,j], e, -9e15))
    out[i]  = attn @ h
Since exp is monotonic and softmax is scale-invariant per row i:
    exp(leaky_relu(s_i+s_j)) * exp(-0.2 s_i) = max(exp(s_j + 0.8 s_i), exp(0.2 s_j))
and the row-constant exp(-0.2 s_i) cancels in the softmax normalization.  With
per-node precomputes rep_i = exp(0.8 s_i) (replicated across partitions),
rv_j = exp(s_j) and v_j = exp(0.2 s_j) (per-partition scalars), the whole
masked softmax numerator for one (j-chunk, head) tile is:
    P'[j,i] = max(rep_i * rv_j, v_j)        one tensor_scalar   (bf16, 4x)
    Pm      = P' * mask[j,i]                one tensor_tensor   (bf16, 2x),
                                            4 heads stacked against a
                                            stride-0-repeat mask AP
— no dense exp/leaky passes on ScalarE at all.  An appended ones-column in the
aggregation operand yields the softmax denominator inside the same PE matmuls
that aggregate h (attention tile stationary, so the output lands
destination-rows-on-partitions and phase 2 is just reciprocal + scale).

Sharding: destination rows i split across 8 cores (512 rows each); every core
computes the full h = x @ [W | W@a_src | W@a_dst] locally (one 128-deep matmul
per j-chunk) and reduces over all 4096 source nodes j for its own rows.

Trn2 scheduling notes: walrus allows at most ONE hardware sync-wait per
engine instruction (extras must be legalized into EventSemaphore ops by
Bacc.finalize, which this kernel relies on).  To keep that legalization
cheap the kernel also ships all bulk inputs as a single concatenated
tensor (one DMA -> one queue semaphore) and drains h PSUM with one engine.
PSUM output accumulators are pre-zeroed with memset and accumulated with
start=False throughout: interleaved per-head accumulation regions sharing
a PSUM bank corrupt each other's first contribution when start=True zeroing
is used per region (observed on HW: last-written head exact, others short).
"""

import os

import numpy as np
import ml_dtypes

import concourse.bass as bass
import concourse.tile as tile
from concourse.bacc import Bacc
from concourse import mybir
from concourse.bass_utils import run_bass_kernel_spmd

bf16 = ml_dtypes.bfloat16

N, IN_DIM, HEADS, OUT_DIM = 4096, 128, 4, 64
NCORES, ROWS = 8, N // 8          # 512 dest rows per core
P = 128                           # partitions
C = N // P                        # 32 j-chunks
OWNC = ROWS // P                  # 4 own i-chunks per core
COLS = 2 * IN_DIM + 2 * HEADS     # 264 = 256 h cols + 4 s_src + 4 s_dst
DAUG = OUT_DIM + 1                # 65: head h-slice + ones column
BULK = ROWS + COLS + N            # xownT | W_aug | xT columns

_cache = {}


def _build_bass(repeat=1, hw_loop=False):
    nc = Bacc()
    f32 = mybir.dt.float32
    f16 = mybir.dt.float16
    bfl = mybir.dt.bfloat16
    Act = mybir.ActivationFunctionType
    Alu = mybir.AluOpType

    bulk = nc.declare_dram_parameter("bulk", [P, BULK], f32, isOutput=False)
    maskT = nc.declare_dram_parameter("maskT", [N, ROWS], bfl, isOutput=False)
    out = nc.declare_dram_parameter("out", [ROWS, HEADS * OUT_DIM], f32, isOutput=True)
    riT_dram = nc.dram_tensor("riT_scratch", [OWNC * HEADS, P], bfl)

    with tile.TileContext(nc) as tc:
        with (
            tc.tile_pool(name="consts", bufs=1) as consts,
            tc.tile_pool(name="hb", bufs=C) as hb_pool,
            tc.tile_pool(name="vr", bufs=C) as vr_pool,
            tc.tile_pool(name="mask", bufs=8) as mask_pool,
            tc.tile_pool(name="tt", bufs=4) as t_pool,
            tc.tile_pool(name="pm", bufs=4) as pm_pool,
            tc.tile_pool(name="fin", bufs=4) as fin_pool,
            tc.tile_pool(name="psout", bufs=1, space="PSUM") as ps_out_pool,
            tc.tile_pool(name="ps_h", bufs=3, space="PSUM") as ps_h_pool,
            tc.tile_pool(name="ps_s", bufs=1, space="PSUM") as ps_s_pool,
        ):
          import contextlib
          loop_ctx = (tc.For_i(0, repeat, 1,
                               hint_engines=tuple(mybir.EngineType(e) for e in
                                                  ("PE", "DVE", "Activation", "SP", "Pool")))
                      if hw_loop else contextlib.nullcontext())
          with loop_ctx:
           for _rep in range(1 if hw_loop else repeat):
            # per-own-chunk output accumulators: claim PSUM banks first so they
            # are never aliased with the h-matmul banks (no cross-pool WAW).
            ps_out = [ps_out_pool.tile([P, HEADS, DAUG], f32, tag=f"po{k}", name=f"ps_out{k}")
                      for k in range(OWNC)]
            for k in range(OWNC):
                nc.vector.memset(ps_out[k][:, :, :], 0.0)

            if os.environ.get("GAT_WARM", "1") == "1":
                # pre-warm the ACT exp table set while input DMAs run
                warm = consts.tile([1, 1], f32, tag="warm")
                nc.vector.memset(warm, 0.0)
                nc.scalar.activation(warm, warm, Act.Exp)

            # ---- all bulk inputs in ONE DMA -> one queue semaphore
            sb_bulk = consts.tile([P, BULK], f32, tag="sb_bulk")
            nc.sync.dma_start(out=sb_bulk[:, 0:ROWS + COLS], in_=bulk[:, 0:ROWS + COLS])
            nc.sync.dma_start(out=sb_bulk[:, ROWS + COLS:BULK], in_=bulk[:, ROWS + COLS:BULK])
            sb_xown = sb_bulk[:, 0:ROWS]
            sb_W = sb_bulk[:, ROWS:ROWS + COLS]
            sb_xT = sb_bulk[:, ROWS + COLS:BULK]
            w_sd = sb_bulk[:, ROWS + 2 * IN_DIM:ROWS + 2 * IN_DIM + HEADS]

            # ---- phase 0b: r_i = exp(0.8 s_src) for own rows, replicated
            # across partitions via DMA transpose + DRAM-bounce broadcast.
            ps_sown = ps_s_pool.tile([P, COLS], f32, tag="ps_s", name="ps_sown")
            for oc in range(OWNC):
                nc.tensor.matmul(
                    ps_sown[:, oc * HEADS:(oc + 1) * HEADS],
                    sb_xown[:, oc * P:(oc + 1) * P], w_sd,
                    start=True, stop=True,
                )
            vown = consts.tile([P, P], bfl, tag="vown")
            nc.vector.memset(vown, 0.0)
            nc.scalar.activation(vown[:, 0:OWNC * HEADS], ps_sown[:, 0:OWNC * HEADS],
                                 Act.Exp, scale=0.8)
            vT = consts.tile([P, P], bfl, tag="vT")
            nc.sync.dma_start(out=vT, in_=vown, transpose=True)
            nc.sync.dma_start(out=riT_dram[:, :], in_=vT[0:OWNC * HEADS, :])
            sb_rep = consts.tile([P, HEADS, ROWS], bfl, tag="sb_rep")
            base = riT_dram[:, :]
            if os.environ.get("GAT_BCAST", "new") == "new":
                for hd in range(HEADS):
                    bcast = bass.AP(tensor=base.tensor, offset=base.offset + hd * P,
                                    ap=[[0, P], [HEADS * P, OWNC], [1, P]])
                    nc.sync.dma_start(
                        out=sb_rep[:, hd, :].rearrange("p (oc t) -> p oc t", oc=OWNC),
                        in_=bcast)
            else:
                for hd in range(HEADS):
                    for oc in range(OWNC):
                        row = riT_dram[oc * HEADS + hd:oc * HEADS + hd + 1, :]
                        b = bass.AP(tensor=row.tensor, offset=row.offset,
                                    ap=[[0, P], row.ap[-1]])
                        nc.sync.dma_start(out=sb_rep[:, hd, oc * P:(oc + 1) * P], in_=b)

            # ---- phase 0c: h_aug per j-chunk; PSUM drained by VectorE only
            hb = []
            vr = []
            for c in range(C):
                ps_h = ps_h_pool.tile([P, COLS], f32, tag="ps_h")
                nc.tensor.matmul(ps_h, sb_xT[:, c * P:(c + 1) * P], sb_W,
                                 start=True, stop=True)
                hb_c = hb_pool.tile([P, HEADS, DAUG], bfl, tag="hb")
                nc.vector.memset(hb_c[:, :, OUT_DIM:DAUG], 1.0)
                nc.scalar.activation(
                    hb_c[:, :, 0:OUT_DIM],
                    ps_h[:, 0:2 * IN_DIM].rearrange("p (h d) -> p h d", h=HEADS),
                    Act.Copy,
                )
                vr_c = vr_pool.tile([P, 2, HEADS], f32, tag="vr")
                nc.scalar.activation(vr_c[:, 0, :], ps_h[:, 2 * IN_DIM + HEADS:COLS],
                                     Act.Exp, scale=0.2)
                nc.scalar.activation(vr_c[:, 1, :], ps_h[:, 2 * IN_DIM + HEADS:COLS],
                                     Act.Exp, scale=1.0)
                hb.append(hb_c)
                vr.append(vr_c)

            # ---- phase 1: hot loop over j-chunks
            for c in range(C):
                mask_c = mask_pool.tile([P, ROWS], bfl, tag="mask")
                nc.sync.dma_start(out=mask_c, in_=maskT[c * P:(c + 1) * P, :])
                t_all = t_pool.tile([P, HEADS, ROWS], bfl, tag="T")
                for hd in range(HEADS):
                    nc.vector.tensor_scalar(
                        out=t_all[:, hd, :], in0=sb_rep[:, hd, :],
                        scalar1=vr[c][:, 1, hd:hd + 1],
                        scalar2=vr[c][:, 0, hd:hd + 1],
                        op0=Alu.mult, op1=Alu.max,
                    )
                pm_all = pm_pool.tile([P, HEADS, ROWS], bfl, tag="pm")
                for hd in range(HEADS):
                    nc.vector.tensor_tensor(out=pm_all[:, hd, :],
                                            in0=t_all[:, hd, :], in1=mask_c,
                                            op=Alu.mult)
                for hd in range(HEADS):
                    for k in range(OWNC):
                        nc.tensor.matmul(
                            ps_out[k][:, hd, :],
                            pm_all[:, hd, k * P:(k + 1) * P], hb[c][:, hd, :],
                            start=False, stop=(c == C - 1),
                            skip_group_check=True,
                        )

            # ---- phase 2: normalize + store (dest rows already on partitions)
            for k in range(OWNC):
                out_k = fin_pool.tile([P, HEADS, OUT_DIM], f32, tag="outk")
                for hd in range(HEADS):
                    rcp = fin_pool.tile([P, 1], f32, tag="rcp")
                    nc.vector.reciprocal(rcp, ps_out[k][:, hd, OUT_DIM:DAUG])
                    if os.environ.get("GAT_FIN", "act") == "act":
                        nc.scalar.activation(
                            out_k[:, hd, :], ps_out[k][:, hd, 0:OUT_DIM],
                            Act.Copy, scale=rcp,
                        )
                    else:
                        nc.vector.tensor_scalar(
                            out=out_k[:, hd, :], in0=ps_out[k][:, hd, 0:OUT_DIM],
                            scalar1=rcp, scalar2=None, op0=Alu.mult,
                        )
                nc.sync.dma_start(
                    out=out[k * P:(k + 1) * P, :].rearrange("p (h d) -> p h d", h=HEADS),
                    in_=out_k,
                )
    nc.finalize()
    return nc


def _prep_in_maps(x, adj_mask, W_lin, a_src, a_dst):

    W_lin = np.asarray(W_lin, np.float32)
    W3 = W_lin.reshape(IN_DIM, HEADS, OUT_DIM).astype(np.float64)
    W_src = (W3 @ np.asarray(a_src, np.float64)).astype(np.float32)
    W_dst = (W3 @ np.asarray(a_dst, np.float64)).astype(np.float32)
    W_aug = np.concatenate([W_lin, W_src, W_dst], axis=1)
    x = np.asarray(x, np.float32)
    xT = np.ascontiguousarray(x.T)
    adj = np.asarray(adj_mask, bool)
    maskT = np.where(adj.T, np.float32(1.0), np.float32(0.0)).astype(bf16)

    in_maps = []
    for core in range(NCORES):
        sl = slice(core * ROWS, (core + 1) * ROWS)
        bulk = np.ascontiguousarray(
            np.concatenate([xT[:, sl], W_aug, xT], axis=1))
        in_maps.append({
            "bulk": bulk,
            "maskT": np.ascontiguousarray(maskT[:, sl]),
        })

    return in_maps


def kernel(x, adj_mask, W_lin, a_src, a_dst):
    if "nc" not in _cache:
        _cache["nc"] = _build_bass()
    nc = _cache["nc"]
    in_maps = _prep_in_maps(x, adj_mask, W_lin, a_src, a_dst)
    res = run_bass_kernel_spmd(nc, in_maps, core_ids=list(range(NCORES)))
    outs = [r["out"] for r in res.results]
    return np.concatenate(outs, axis=0).astype(np.float32)



# revision 5
# speedup vs baseline: 5.1335x; 5.1335x over previous
"""DenseGATv2 layer on 8 Trainium2 NeuronCores (Bass/Tile).

Math: the reference computes, per head,
    e[i,j]  = leaky_relu(s_i[i] + s_j[j], 0.2)   (s_i = h@a_src, s_j = h@a_dst)
    attn    = softmax_j(where(adj[i,j], e, -9e15))
    out[i]  = attn @ h
Since exp is monotonic and softmax is scale-invariant per row i:
    exp(leaky_relu(s_i+s_j)) * exp(-0.2 s_i) = max(exp(s_j + 0.8 s_i), exp(0.2 s_j))
and the row-constant exp(-0.2 s_i) cancels in the softmax normalization.  With
rep_i = exp(0.8 s_i) (replicated across partitions), rv_j = exp(s_j) and
v_j = exp(0.2 s_j), the masked softmax numerator is
    Pm[j,i] = max(rep_i * rv_j, v_j) * mask[j,i]
and an appended ones-column in the stationary aggregation operand yields the
softmax denominator inside the same PE matmuls that aggregate h.

Execution-cost model (measured on this axon platform): every engine
instruction costs a near-flat ~60-100us regardless of operand size (matmul
~95us for 65 or 512 moving cols alike, DVE ~60-90us from 512 to 32768 cols,
DMA ~15us even for multi-MB transfers), and engines do not overlap.  So the
kernel minimizes INSTRUCTION COUNT:
  - one 4MB mask DMA (strided into [128, 32 chunks, 512]),
  - h for all 4096 nodes: 32 matmuls into a 4-bank PSUM tile, drained by 8
    multi-bank copy-activations; per-head ones-columns by 1 strided memset;
    all per-node exps (rv, v) by 1 strided mega-activation,
  - scores: 3 giant tensor_tensor ops per 8-chunk super-chunk (stride-0
    broadcast APs for rep/rv/v/mask) = 12 DVE instrs total,
  - aggregation: h stationary [j,65], attention moving [j, 512 dest rows]
    = 4 heads x 32 chunks = 128 matmuls accumulating into 4 PSUM banks
    (output lands transposed: [head-dim, dest rows]),
  - normalize+untranspose: 4 drain acts, transpose via a DRAM bounce with
    permuted-stride DMA APs (f32-safe, unlike the 2-byte xbar path),
    1 reciprocal, 1 broadcast multiply, 1 output DMA.

Sharding: destination rows i split across 8 cores (512 rows each); every core
computes the full h = x @ [Wh0|0|..|Wh3|0|W@a_dst|0.2 W@a_dst] locally and
reduces over all 4096 source nodes j for its own rows.  Per-core own-row
rep_i comes from 4 extra matmuls on a host-sharded xT_own slice, one exp,
and a transposing DRAM bounce + stride-0 broadcast DMA.
"""

import numpy as np
import ml_dtypes

import concourse.bass as bass
import concourse.tile as tile
from concourse.bacc import Bacc
from concourse import mybir
from concourse.bass_utils import run_bass_kernel_spmd

bf16 = ml_dtypes.bfloat16

N, IN_DIM, HEADS, OUT_DIM = 4096, 128, 4, 64
NCORES, ROWS = 8, N // 8          # 512 dest rows per core
P = 128                           # partitions
C = N // P                        # 32 j-chunks
OWNC = ROWS // P                  # 4 own i-chunks per core
DAUG = OUT_DIM + 1                # 65: head h-slice + ones column
WCOLS = HEADS * DAUG + 2 * HEADS  # 268: 4x(64 h + 1 zero) + W_dst + 0.2*W_dst
BULK = WCOLS + HEADS + ROWS + N   # 4880: W_aug | W_rep | xT_own | xT
SUP = 8                           # chunks per super-chunk in the score phase
NSUP = C // SUP

_cache = {}


def _fap(apobj, free_dims, extra_offset=0):
    """AP keeping the real partition entry, custom free dims [[stride, count],...]."""
    return bass.AP(tensor=apobj.tensor, offset=apobj.offset + extra_offset,
                   ap=[list(apobj.ap[0])] + [list(d) for d in free_dims])


def _build_bass(repeat=1):
    nc = Bacc()
    f32 = mybir.dt.float32
    bfl = mybir.dt.bfloat16
    Act = mybir.ActivationFunctionType
    Alu = mybir.AluOpType

    bulk = nc.declare_dram_parameter("bulk", [P, BULK], f32, isOutput=False)
    maskT = nc.declare_dram_parameter("maskT", [N, ROWS], bfl, isOutput=False)
    out = nc.declare_dram_parameter("out", [ROWS, HEADS * OUT_DIM], f32, isOutput=True)
    riT = nc.dram_tensor("riT_scratch", [OWNC * HEADS, P], f32)        # [(oc,h), q]
    p2 = nc.dram_tensor("p2_scratch", [ROWS, HEADS * DAUG], f32)       # [i, (h,d)]

    with tile.TileContext(nc) as tc:
        with (
            tc.tile_pool(name="consts", bufs=1) as consts,
            tc.tile_pool(name="tt", bufs=1) as t_pool,
            tc.tile_pool(name="fin", bufs=1) as fin_pool,
            tc.tile_pool(name="psout", bufs=1, space="PSUM") as ps_out_pool,
            tc.tile_pool(name="ps_h", bufs=1, space="PSUM") as ps_h_pool,
        ):
            for _rep in range(repeat):
                # claim the 4 aggregation banks first so they never alias ps_h
                ps_out = [ps_out_pool.tile([DAUG, ROWS], f32, tag=f"po{h}",
                                           name=f"ps_out{h}") for h in range(HEADS)]

                sb_bulk = consts.tile([P, BULK], f32, tag="sb_bulk")
                nc.sync.dma_start(out=sb_bulk[:, 0:WCOLS + HEADS + ROWS],
                                  in_=bulk[:, 0:WCOLS + HEADS + ROWS])
                nc.sync.dma_start(out=sb_bulk[:, WCOLS + HEADS + ROWS:BULK],
                                  in_=bulk[:, WCOLS + HEADS + ROWS:BULK])
                sb_W = sb_bulk[:, 0:WCOLS]
                sb_Wrep = sb_bulk[:, WCOLS:WCOLS + HEADS]
                sb_xown = sb_bulk[:, WCOLS + HEADS:WCOLS + HEADS + ROWS]
                sb_xT = sb_bulk[:, WCOLS + HEADS + ROWS:BULK]

                # mask for all chunks in one strided DMA: [p, c, i] <- maskT[c*128+p, i]
                mask_all = consts.tile([P, C, ROWS], bfl, tag="mask")
                mT = maskT[:, :]
                nc.sync.dma_start(out=mask_all, in_=bass.AP(
                    tensor=mT.tensor, offset=mT.offset,
                    ap=[[ROWS, P], [P * ROWS, C], [1, ROWS]]))

                # ---- own-row rep_i = exp(0.8 s_src): 1 matmul (W_rep stationary
                # -> [h, i] layout directly), 1 exp, DRAM bounce + 1 bcast DMA
                ps0 = ps_h_pool.tile([P, 4, 512], f32, tag="ps4", name="ps_own")
                nc.tensor.matmul(ps0[0:HEADS, 0, :], sb_Wrep, sb_xown,
                                 start=True, stop=True)
                vownT = consts.tile([HEADS, ROWS], f32, tag="vownT")
                nc.scalar.activation(vownT, ps0[0:HEADS, 0, :], Act.Exp)
                nc.sync.dma_start(out=riT[:, :], in_=vownT)
                rep_sb = consts.tile([P, HEADS, ROWS], f32, tag="rep")
                nc.sync.dma_start(
                    out=rep_sb,
                    in_=bass.AP(tensor=riT[:, :].tensor, offset=0,
                                ap=[[0, P], [ROWS, HEADS], [1, ROWS]]))

                # ---- h_aug for all 4096 nodes: 32 matmuls, 8 multi-bank drains
                big = consts.tile([P, C, WCOLS], f32, tag="big")
                for g in range(C // 4):
                    psg = ps_h_pool.tile([P, 4, 512], f32, tag="ps4", name=f"ps_h{g}")
                    for k in range(4):
                        nc.tensor.matmul(psg[:, k, 0:WCOLS],
                                         sb_xT[:, (g * 4 + k) * P:(g * 4 + k + 1) * P],
                                         sb_W, start=True, stop=True)
                    nc.scalar.activation(
                        big[:, g * 4:(g + 1) * 4, :],
                        _fap(psg[:, :, :], [[512, 4], [1, WCOLS]]),
                        Act.Copy)
                # ones columns (denominator) + all per-node exps, one instr each
                biga = big[:, :, :]
                nc.vector.memset(
                    _fap(biga, [[WCOLS, C], [DAUG, HEADS]], extra_offset=OUT_DIM), 1.0)
                rvv = consts.tile([P, C, 2 * HEADS], f32, tag="rvv")
                nc.scalar.activation(
                    rvv, _fap(biga, [[WCOLS, C], [1, 2 * HEADS]],
                              extra_offset=HEADS * DAUG),
                    Act.Exp)

                # ---- scores + aggregation: 3 giant TTs + 32 matmuls per super
                rep_a = rep_sb[:, :, :]
                rvv_a = rvv[:, :, :]
                mask_a = mask_all[:, :, :]
                for s in range(NSUP):
                    t = t_pool.tile([P, SUP, HEADS, ROWS], f32, tag="t")
                    ta = t[:, :, :, :]
                    nc.vector.tensor_tensor(
                        out=ta,
                        in0=_fap(rep_a, [[0, SUP], [ROWS, HEADS], [1, ROWS]]),
                        in1=_fap(rvv_a, [[2 * HEADS, SUP], [1, HEADS], [0, ROWS]],
                                 extra_offset=s * SUP * 2 * HEADS),
                        op=Alu.mult)
                    nc.vector.tensor_tensor(
                        out=ta, in0=ta,
                        in1=_fap(rvv_a, [[2 * HEADS, SUP], [1, HEADS], [0, ROWS]],
                                 extra_offset=s * SUP * 2 * HEADS + HEADS),
                        op=Alu.max)
                    nc.vector.tensor_tensor(
                        out=ta, in0=ta,
                        in1=_fap(mask_a, [[ROWS, SUP], [0, HEADS], [1, ROWS]],
                                 extra_offset=s * SUP * ROWS),
                        op=Alu.mult)
                    for c8 in range(SUP):
                        c = s * SUP + c8
                        for hd in range(HEADS):
                            nc.tensor.matmul(
                                ps_out[hd][:, :],
                                big[:, c, hd * DAUG:(hd + 1) * DAUG],
                                t[:, c8, hd, :],
                                start=(c == 0), stop=(c == C - 1),
                                skip_group_check=True)

                # ---- normalize + untranspose via DRAM bounce
                fin = fin_pool.tile([DAUG, HEADS, ROWS], f32, tag="fin")
                for hd in range(HEADS):
                    nc.scalar.activation(fin[:, hd, :], ps_out[hd][:, :], Act.Copy)
                # p2[i, (h,d)] = fin[d, h, i]  (per-head: DMA APs cap at 3 dims)
                for hd in range(HEADS):
                    nc.sync.dma_start(
                        out=bass.AP(tensor=p2[:, :].tensor, offset=hd * DAUG,
                                    ap=[[1, DAUG], [HEADS * DAUG, ROWS]]),
                        in_=fin[:, hd, :])
                tr = fin_pool.tile([P, OWNC, HEADS, DAUG], f32, tag="tr")
                for hd in range(HEADS):
                    nc.sync.dma_start(
                        out=tr[:, :, hd, :],
                        in_=bass.AP(tensor=p2[:, :].tensor, offset=hd * DAUG,
                                    ap=[[HEADS * DAUG, P],
                                        [HEADS * DAUG * P, OWNC], [1, DAUG]]))
                rcp = fin_pool.tile([P, OWNC, HEADS], f32, tag="rcp")
                tra = tr[:, :, :, :]
                nc.vector.reciprocal(
                    rcp, _fap(tra, [[HEADS * DAUG, OWNC], [DAUG, HEADS]],
                              extra_offset=OUT_DIM))
                out_t = fin_pool.tile([P, OWNC, HEADS, OUT_DIM], f32, tag="outt")
                rcp_a = rcp[:, :, :]
                nc.vector.tensor_tensor(
                    out=out_t,
                    in0=_fap(tra, [[HEADS * DAUG, OWNC], [DAUG, HEADS], [1, OUT_DIM]]),
                    in1=_fap(rcp_a, [[HEADS, OWNC], [1, HEADS], [0, OUT_DIM]]),
                    op=Alu.mult)
                oa = out[:, :]
                for oc in range(OWNC):
                    nc.sync.dma_start(
                        out=bass.AP(tensor=oa.tensor,
                                    offset=oa.offset + oc * P * HEADS * OUT_DIM,
                                    ap=[[HEADS * OUT_DIM, P],
                                        [1, HEADS * OUT_DIM]]),
                        in_=out_t[:, oc, :, :])
    nc.finalize()
    return nc


def _prep_in_maps(x, adj_mask, W_lin, a_src, a_dst):
    W_lin = np.asarray(W_lin, np.float32)
    W3 = W_lin.reshape(IN_DIM, HEADS, OUT_DIM).astype(np.float64)
    W_src = (W3 @ np.asarray(a_src, np.float64)).astype(np.float32)   # [128, 4]
    W_dst = (W3 @ np.asarray(a_dst, np.float64)).astype(np.float32)   # [128, 4]

    W_aug = np.zeros((IN_DIM, WCOLS), np.float32)
    for h in range(HEADS):
        W_aug[:, h * DAUG:h * DAUG + OUT_DIM] = W_lin[:, h * OUT_DIM:(h + 1) * OUT_DIM]
    W_aug[:, HEADS * DAUG:HEADS * DAUG + HEADS] = W_dst
    W_aug[:, HEADS * DAUG + HEADS:WCOLS] = 0.2 * W_dst

    x = np.asarray(x, np.float32)
    xT = np.ascontiguousarray(x.T)
    adj = np.asarray(adj_mask, bool)
    maskT = np.where(adj.T, np.float32(1.0), np.float32(0.0)).astype(bf16)

    in_maps = []
    for core in range(NCORES):
        sl = slice(core * ROWS, (core + 1) * ROWS)
        bulk = np.ascontiguousarray(
            np.concatenate([W_aug, 0.8 * W_src, xT[:, sl], xT], axis=1))
        in_maps.append({
            "bulk": bulk,
            "maskT": np.ascontiguousarray(maskT[:, sl]),
        })
    return in_maps


def kernel(x, adj_mask, W_lin, a_src, a_dst):
    if "nc" not in _cache:
        _cache["nc"] = _build_bass()
    nc = _cache["nc"]
    in_maps = _prep_in_maps(x, adj_mask, W_lin, a_src, a_dst)
    res = run_bass_kernel_spmd(nc, in_maps, core_ids=list(range(NCORES)))
    outs = [r["out"] for r in res.results]
    return np.concatenate(outs, axis=0).astype(np.float32)
